# revision 1
# baseline (speedup 1.0000x reference)
"""Trainium2 Bass kernel for a 2-layer FC-LSTM (B=512, T=128, D=300, H=1024).

Strategy: model-parallel over the gate/hidden dimension. Each of the 8
cores owns 128 hidden units per layer (512 gate rows), keeps its weight
slices resident in SBUF, and computes gates in transposed layout
[gates, batch] so every matmul is M=128 x K=128 x N=512 (float32r, full
PE throughput). The hidden states live transposed h.T = [H, B]; after a
core computes its 128-row slice of h.T it AllGathers the full [1024, B]
h.T for the next step's recurrent matmuls. The mean-over-time decoder is
folded into a final per-core partial matvec that the host sums.
"""
import sys

sys.path.insert(0, "/opt/trn_rl_repo")

import os
import numpy as np

import concourse.bass as bass
import concourse.bacc as bacc
import concourse.mybir as mybir
from concourse import tile
from concourse.bass_utils import run_bass_kernel_spmd

B, T, D, H = 512, 128, 300, 1024
NCORES = 8
HL = H // NCORES          # 128 hidden units owned per core (per layer)
GL = 4 * HL               # 512 gate rows owned per core
DK = [128, 128, 44]       # D=300 split into K-chunks
KH = H // 128             # 8 K-chunks over the hidden dim

F32 = mybir.dt.float32
F32R = mybir.dt.float32r
AF = mybir.ActivationFunctionType
_NO_COLL = bool(os.environ.get("KERNEL_NO_COLL"))
ALU = mybir.AluOpType

def _build(t_steps, t_total=None):
    t_total = t_total or t_steps
    nc = bacc.Bacc("TRN2", target_bir_lowering=False, debug=False, num_devices=NCORES)

    xT = nc.dram_tensor("xT", [t_total, D, B], F32R, kind="ExternalInput")
    w0x = nc.dram_tensor("w0x", [128, 3 * GL], F32R, kind="ExternalInput")
    w0h = nc.dram_tensor("w0h", [128, KH * GL], F32R, kind="ExternalInput")
    w1x = nc.dram_tensor("w1x", [128, KH * GL], F32R, kind="ExternalInput")
    w1h = nc.dram_tensor("w1h", [128, KH * GL], F32R, kind="ExternalInput")
    b0d = nc.dram_tensor("b0d", [HL, 4], F32, kind="ExternalInput")
    b1d = nc.dram_tensor("b1d", [HL, 4], F32, kind="ExternalInput")
    wdec = nc.dram_tensor("wdec", [HL, 1], F32R, kind="ExternalInput")
    out_p = nc.dram_tensor("out_p", [1, B], F32, kind="ExternalOutput")

    rg = [list(range(NCORES))]

    with tile.TileContext(nc) as tc:
        with (
            tc.tile_pool(name="wpool", bufs=1) as wp,
            tc.tile_pool(name="hpool", bufs=1) as hp,
            tc.tile_pool(name="xpool", bufs=3) as xp,
            tc.tile_pool(name="zpool", bufs=1) as zp,
            tc.tile_pool(name="cpool", bufs=2) as cp,
            tc.tile_pool(name="pp", bufs=1, space="PSUM") as pp,
            tc.tile_pool(name="dram", bufs=2, space="DRAM") as dp,
        ):
            w0x_s = wp.tile([128, 3 * GL], F32R, tag="w0x", name="w0x")
            nc.sync.dma_start(w0x_s[:], w0x.ap())
            w0h_s = wp.tile([128, KH * GL], F32R, tag="w0h", name="w0h")
            nc.sync.dma_start(w0h_s[:], w0h.ap())
            w1x_s = wp.tile([128, KH * GL], F32R, tag="w1x", name="w1x")
            nc.sync.dma_start(w1x_s[:], w1x.ap())
            w1h_s = wp.tile([128, KH * GL], F32R, tag="w1h", name="w1h")
            nc.sync.dma_start(w1h_s[:], w1h.ap())
            b0_s = wp.tile([HL, 4], F32, tag="b0", name="b0")
            nc.sync.dma_start(b0_s[:], b0d.ap())
            b1_s = wp.tile([HL, 4], F32, tag="b1", name="b1")
            nc.sync.dma_start(b1_s[:], b1d.ap())
            wdec_s = wp.tile([HL, 1], F32R, tag="wdec", name="wdec")
            nc.sync.dma_start(wdec_s[:], wdec.ap())

            def wx_lhsT(kc, m):
                kp = DK[kc]
                return w0x_s[0:kp, kc * GL + m * 128 : kc * GL + (m + 1) * 128]

            def wh_lhsT(w_s, k, m):
                return w_s[0:128, k * GL + m * 128 : k * GL + (m + 1) * 128]

            c0_prev = c1_prev = None
            h0T = h1T = None
            acc = None

            for t in range(t_steps):
                # -- x_t.T loaded as 3 K-chunk tiles [*, B] --
                xt = xp.tile([128, 3 * B], F32R, tag="xt", name="xt")
                for kc in range(3):
                    nc.sync.dma_start(
                        xt[0 : DK[kc], kc * B : (kc + 1) * B],
                        xT.ap()[t, kc * 128 : kc * 128 + DK[kc], :],
                    )

                # -- layer0 matmuls: gates0.T[m] = W0x.T@xt + W0h.T@h0T --
                ps0 = [pp.tile([128, B], F32, tag=f"ps0{g}", name=f"ps0{g}") for g in range(4)]
                for m in range(4):
                    for kc in range(3):
                        nc.tensor.matmul(
                            ps0[m][:],
                            wx_lhsT(kc, m),
                            xt[0 : DK[kc], kc * B : (kc + 1) * B],
                            start=(kc == 0),
                            stop=(t == 0 and kc == 2),
                        )
                if t > 0:
                    for m in range(4):
                        for k in range(KH):
                            nc.tensor.matmul(
                                ps0[m][:],
                                wh_lhsT(w0h_s, k, m),
                                h0T[:, k * B : (k + 1) * B],
                                start=False,
                                stop=(k == KH - 1),
                            )

                # -- layer0 cell --
                zi = zp.tile([128, B], F32, tag="zi", name="zi")
                zf = zp.tile([128, B], F32, tag="zf", name="zf")
                zg = zp.tile([128, B], F32, tag="zg", name="zg")
                zo = zp.tile([128, B], F32, tag="zo", name="zo")
                nc.scalar.activation(zi[:], ps0[0][:], AF.Sigmoid, bias=b0_s[:, 0:1])
                if t > 0:
                    nc.scalar.activation(zf[:], ps0[1][:], AF.Sigmoid, bias=b0_s[:, 1:2])
                nc.scalar.activation(zg[:], ps0[2][:], AF.Tanh, bias=b0_s[:, 2:3])
                nc.scalar.activation(zo[:], ps0[3][:], AF.Sigmoid, bias=b0_s[:, 3:4])
                c0 = cp.tile([128, B], F32, tag="c0", name="c0")
                if t == 0:
                    nc.vector.tensor_mul(c0[:], zi[:], zg[:])
                else:
                    ca = zp.tile([128, B], F32, tag="ca", name="ca")
                    cb = zp.tile([128, B], F32, tag="cb", name="cb")
                    nc.vector.tensor_mul(ca[:], zf[:], c0_prev[:])
                    nc.vector.tensor_mul(cb[:], zi[:], zg[:])
                    nc.vector.tensor_add(c0[:], ca[:], cb[:])
                c0_prev = c0
                tc0 = zp.tile([128, B], F32, tag="tc0", name="tc0")
                nc.scalar.activation(tc0[:], c0[:], AF.Tanh)
                h0loc = zp.tile([128, B], F32R, tag="h0loc", name="h0loc")
                nc.vector.tensor_mul(h0loc[:], zo[:], tc0[:])

                # -- AllGather h0.T --
                ag0i = dp.tile([HL, B], F32R, tag="ag0i", name="ag0i")
                ag0o = dp.tile([H, B], F32R, tag="ag0o", name="ag0o")
                nc.gpsimd.dma_start(ag0i[:], h0loc[:])
                if not _NO_COLL:
                    nc.gpsimd.collective_compute(
                        "AllGather", ALU.bypass, replica_groups=rg,
                        ins=[ag0i.opt().bitcast(F32)], outs=[ag0o.opt().bitcast(F32)],
                    )
                else:
                    for _k in range(KH):
                        nc.gpsimd.dma_start(ag0o[_k * 128 : (_k + 1) * 128, :], ag0i[:])
                h0T = hp.tile([128, KH * B], F32R, tag="h0T", name="h0T")
                for k in range(KH):
                    nc.gpsimd.dma_start(
                        h0T[:, k * B : (k + 1) * B], ag0o[k * 128 : (k + 1) * 128, :]
                    )

                # -- layer1 matmuls: gates1.T[m] = W1h.T@h1T + W1x.T@h0T --
                ps1 = [pp.tile([128, B], F32, tag=f"ps1{g}", name=f"ps1{g}") for g in range(4)]
                if t > 0:
                    for m in range(4):
                        for k in range(KH):
                            nc.tensor.matmul(
                                ps1[m][:],
                                wh_lhsT(w1h_s, k, m),
                                h1T[:, k * B : (k + 1) * B],
                                start=(k == 0),
                                stop=False,
                            )
                for m in range(4):
                    for k in range(KH):
                        nc.tensor.matmul(
                            ps1[m][:],
                            wh_lhsT(w1x_s, k, m),
                            h0T[:, k * B : (k + 1) * B],
                            start=(t == 0 and k == 0),
                            stop=(k == KH - 1),
                        )

                # -- layer1 cell --
                yi = zp.tile([128, B], F32, tag="yi", name="yi")
                yf = zp.tile([128, B], F32, tag="yf", name="yf")
                yg = zp.tile([128, B], F32, tag="yg", name="yg")
                yo = zp.tile([128, B], F32, tag="yo", name="yo")
                nc.scalar.activation(yi[:], ps1[0][:], AF.Sigmoid, bias=b1_s[:, 0:1])
                if t > 0:
                    nc.scalar.activation(yf[:], ps1[1][:], AF.Sigmoid, bias=b1_s[:, 1:2])
                nc.scalar.activation(yg[:], ps1[2][:], AF.Tanh, bias=b1_s[:, 2:3])
                nc.scalar.activation(yo[:], ps1[3][:], AF.Sigmoid, bias=b1_s[:, 3:4])
                c1 = cp.tile([128, B], F32, tag="c1", name="c1")
                if t == 0:
                    nc.vector.tensor_mul(c1[:], yi[:], yg[:])
                else:
                    da = zp.tile([128, B], F32, tag="da", name="da")
                    db = zp.tile([128, B], F32, tag="db", name="db")
                    nc.vector.tensor_mul(da[:], yf[:], c1_prev[:])
                    nc.vector.tensor_mul(db[:], yi[:], yg[:])
                    nc.vector.tensor_add(c1[:], da[:], db[:])
                c1_prev = c1
                tc1 = zp.tile([128, B], F32, tag="tc1", name="tc1")
                nc.scalar.activation(tc1[:], c1[:], AF.Tanh)
                h1loc = zp.tile([128, B], F32R, tag="h1loc", name="h1loc")
                nc.vector.tensor_mul(h1loc[:], yo[:], tc1[:])

                # -- acc += h1 --
                if t == 0:
                    acc = cp.tile([128, B], F32R, tag="acc", name="acc")
                    nc.vector.tensor_copy(acc[:], h1loc[:])
                else:
                    acc_new = cp.tile([128, B], F32R, tag="acc", name="acc")
                    nc.vector.tensor_add(acc_new[:], acc[:], h1loc[:])
                    acc = acc_new

                # -- AllGather h1.T (skip on the last step: not needed again) --
                if t < t_steps - 1:
                    ag1i = dp.tile([HL, B], F32R, tag="ag1i", name="ag1i")
                    ag1o = dp.tile([H, B], F32R, tag="ag1o", name="ag1o")
                    nc.gpsimd.dma_start(ag1i[:], h1loc[:])
                    if not _NO_COLL:
                        nc.gpsimd.collective_compute(
                            "AllGather", ALU.bypass, replica_groups=rg,
                            ins=[ag1i.opt().bitcast(F32)], outs=[ag1o.opt().bitcast(F32)],
                        )
                    else:
                        for _k in range(KH):
                            nc.gpsimd.dma_start(ag1o[_k * 128 : (_k + 1) * 128, :], ag1i[:])
                    h1T = hp.tile([128, KH * B], F32R, tag="h1T", name="h1T")
                    for k in range(KH):
                        nc.gpsimd.dma_start(
                            h1T[:, k * B : (k + 1) * B], ag1o[k * 128 : (k + 1) * 128, :]
                        )

            # -- decoder partial: out_p = (acc/T) . wdec (host sums cores) --
            psd = pp.tile([128, B], F32, tag="ps10", name="ps10")
            nc.tensor.matmul(
                psd[0:1, :], wdec_s[:, 0:1], acc[:],
                start=True, stop=True,
            )
            outt = zp.tile([1, B], F32, tag="outt", name="outt")
            nc.scalar.copy(outt[:], psd[0:1, :])
            nc.sync.dma_start(out_p.ap(), outt[:])

    nc.compile()
    return nc


def _prep_inputs(x, W_ih0, W_hh0, b_ih0, b_hh0, W_ih1, W_hh1, b_ih1, b_hh1, W_dec, t_steps, t_total=None):
    t_total = t_total or t_steps
    xT = np.ascontiguousarray(np.transpose(x[:, :t_total, :], (1, 2, 0))).astype(np.float32)
    b0 = (b_ih0 + b_hh0).astype(np.float32)
    b1 = (b_ih1 + b_hh1).astype(np.float32)
    in_maps = []
    for c in range(NCORES):
        rows = np.concatenate([g * H + np.arange(c * HL, (c + 1) * HL) for g in range(4)])

        def pack(W, nk):
            Wt = np.ascontiguousarray(W[rows, :].T.astype(np.float32))  # [K_total, GL]
            arr = np.zeros((128, nk * GL), np.float32)
            for k in range(nk):
                kp = min(128, Wt.shape[0] - k * 128)
                arr[0:kp, k * GL : k * GL + GL] = Wt[k * 128 : k * 128 + kp, :]
            return arr

        in_maps.append({
            "xT": xT,
            "w0x": pack(W_ih0, 3),
            "w0h": pack(W_hh0, KH),
            "w1x": pack(W_ih1, KH),
            "w1h": pack(W_hh1, KH),
            "b0d": np.ascontiguousarray(b0[rows].reshape(4, HL).T),
            "b1d": np.ascontiguousarray(b1[rows].reshape(4, HL).T),
            "wdec": np.ascontiguousarray(
                (W_dec[0, c * HL : (c + 1) * HL] / np.float32(t_steps)).reshape(HL, 1)
            ).astype(np.float32),
        })
    return in_maps


def _run(inputs, t_steps, **spmd_kwargs):
    nc = _build(t_steps)
    in_maps = _prep_inputs(
        inputs["x"], inputs["W_ih0"], inputs["W_hh0"], inputs["b_ih0"], inputs["b_hh0"],
        inputs["W_ih1"], inputs["W_hh1"], inputs["b_ih1"], inputs["b_hh1"], inputs["W_dec"],
        t_steps,
    )
    res = run_bass_kernel_spmd(nc, in_maps, core_ids=list(range(NCORES)), **spmd_kwargs)
    part = sum(res.results[c]["out_p"][0] for c in range(NCORES))
    out = (part + inputs["b_dec"][0]).astype(np.float32).reshape(B, 1)
    return out, res


def _bench(inputs, t_steps, n_timed=3, t_total=None):
    """Build once, then run the cached PJRT executable repeatedly on
    device-resident inputs, returning (out, per-run wall seconds list).

    Mirrors bass2jax.run_bass_via_pjrt's multi-core branch but without
    output donation so the executable can be re-invoked for timing.
    """
    import time
    import jax
    from jax.experimental.shard_map import shard_map
    from jax.sharding import Mesh, PartitionSpec, NamedSharding
    from concourse import bass2jax
    from concourse import mybir as _mybir

    nc = _build(t_steps, t_total)
    in_maps = _prep_inputs(
        inputs["x"], inputs["W_ih0"], inputs["W_hh0"], inputs["b_ih0"], inputs["b_hh0"],
        inputs["W_ih1"], inputs["W_hh1"], inputs["b_ih1"], inputs["b_hh1"], inputs["W_dec"],
        t_steps, t_total,
    )
    bass2jax.install_neuronx_cc_hook()

    partition_name = nc.partition_id_tensor.name if nc.partition_id_tensor else None
    in_names, out_names, out_avals, zero_outs = [], [], [], []
    for alloc in nc.m.functions[0].allocations:
        if not isinstance(alloc, _mybir.MemoryLocationSet):
            continue
        name = alloc.memorylocations[0].name
        if alloc.kind == "ExternalInput":
            if name != partition_name:
                in_names.append(name)
        elif alloc.kind == "ExternalOutput":
            shape = tuple(alloc.tensor_shape)
            dtype = _mybir.dt.np(alloc.dtype)
            out_names.append(name)
            out_avals.append(jax.core.ShapedArray(shape, dtype))
            zero_outs.append(np.zeros(shape, dtype))
    n_params = len(in_names)
    all_in_names = list(in_names) + list(out_names)
    if partition_name is not None:
        all_in_names.append(partition_name)

    def _body(*args):
        operands = list(args)
        if partition_name is not None:
            operands.append(bass2jax.partition_id_tensor())
        outs = bass2jax._bass_exec_p.bind(
            *operands,
            out_avals=tuple(out_avals),
            in_names=tuple(all_in_names),
            out_names=tuple(out_names),
            lowering_input_output_aliases=(),
            sim_require_finite=True,
            sim_require_nnan=True,
            nc=nc,
        )
        return tuple(outs)

    devices = jax.devices()[:NCORES]
    mesh = Mesh(np.asarray(devices), ("core",))
    spec = PartitionSpec("core")
    n_outs = len(out_names)
    sharded = jax.jit(
        shard_map(_body, mesh=mesh, in_specs=(spec,) * (n_params + n_outs),
                  out_specs=(spec,) * n_outs, check_rep=False),
        keep_unused=True,
    )
    sharding = NamedSharding(mesh, spec)
    dev_args = []
    for i, name in enumerate(in_names):
        cat = np.concatenate([np.asarray(in_maps[c][name]) for c in range(NCORES)], axis=0)
        dev_args.append(jax.device_put(cat, sharding))
    for z in zero_outs:
        cat = np.zeros((NCORES * z.shape[0], *z.shape[1:]), z.dtype)
        dev_args.append(jax.device_put(cat, sharding))

    out_arrs = sharded(*dev_args)  # compile + warmup
    jax.block_until_ready(out_arrs)
    times = []
    for _ in range(n_timed):
        t0 = time.perf_counter()
        out_arrs = sharded(*dev_args)
        jax.block_until_ready(out_arrs)
        times.append(time.perf_counter() - t0)

    parts = np.asarray(out_arrs[out_names.index("out_p")]).reshape(NCORES, 1, B)
    part = parts.sum(axis=0)[0]
    out = (part + inputs["b_dec"][0]).astype(np.float32).reshape(B, 1)
    return out, times


def kernel(**inputs):
    out, _ = _run(inputs, T)
    return out



# revision 3
# speedup vs baseline: 1.5500x; 1.5500x over previous
"""Trainium2 Bass kernel for a 2-layer FC-LSTM (B=512, T=128, D=300, H=1024).

Strategy: model-parallel over the hidden dim (each of 8 cores owns 128
hidden units per layer = 512 gate rows), with the batch processed as two
independent 256-sample halves that are software-pipelined against each
other. Weights and activations are bf16 (fp32 PSUM accumulation and fp32
cell state), so matmuls run at the PE's full 1 cycle/row rate.

Per half and step a single fused AllGather moves [h0(t); h1(t-1)] through
a Shared-output HBM buffer; it is issued ~one full iteration before its
consumers, so collective latency hides under the other half's matmuls.
All data DMAs ride the sync-engine hardware DGE queue (fused 3D access
patterns: one DMA per x-step prefetch, one per gathered h tensor).
"""
import sys

sys.path.insert(0, "/opt/trn_rl_repo")

import os
import numpy as np

import concourse.bass as bass
import concourse.bacc as bacc
import concourse.mybir as mybir
from concourse import tile
from concourse.bass_utils import run_bass_kernel_spmd

B, T, D, H = 512, 128, 300, 1024
NCORES = 8
NH = 2                    # batch halves (independent recurrences)
Bh = B // NH              # 256 batch per half
HL = H // NCORES          # 128 hidden units owned per core (per layer)
GL = 4 * HL               # 512 gate rows owned per core
DK = [128, 128, 44]       # D=300 split into K-chunks
KH = H // 128             # 8 K-chunks over the hidden dim

F32 = mybir.dt.float32
F32R = mybir.dt.float32r
BF16 = mybir.dt.bfloat16
AF = mybir.ActivationFunctionType
ALU = mybir.AluOpType
_NO_COLL = bool(os.environ.get("KERNEL_NO_COLL"))


def _build(t_steps, t_total=None):
    t_total = t_total or t_steps
    nc = bacc.Bacc("TRN2", target_bir_lowering=False, debug=False, num_devices=NCORES)

    # x pre-transposed+padded on host: xTv[t, p, kc, b] = x[b, t, kc*128+p]
    xTv = nc.dram_tensor("xTv", [t_total, 128, 3, B], BF16, kind="ExternalInput")
    w0x = nc.dram_tensor("w0x", [128, 3 * GL], BF16, kind="ExternalInput")
    w0h = nc.dram_tensor("w0h", [128, KH * GL], BF16, kind="ExternalInput")
    w1x = nc.dram_tensor("w1x", [128, KH * GL], BF16, kind="ExternalInput")
    w1h = nc.dram_tensor("w1h", [128, KH * GL], BF16, kind="ExternalInput")
    b0d = nc.dram_tensor("b0d", [HL, 4], F32, kind="ExternalInput")
    b1d = nc.dram_tensor("b1d", [HL, 4], F32, kind="ExternalInput")
    wdec = nc.dram_tensor("wdec", [HL, 1], F32R, kind="ExternalInput")
    out_p = nc.dram_tensor("out_p", [1, B], F32, kind="ExternalOutput")

    rg = [list(range(NCORES))]

    with tile.TileContext(nc) as tc:
        with (
            tc.tile_pool(name="wpool", bufs=1) as wp,
            tc.tile_pool(name="xpool", bufs=2) as xp,
            tc.tile_pool(name="hpool", bufs=2) as hp,
            tc.tile_pool(name="zpool", bufs=2) as zp,
            tc.tile_pool(name="cpool", bufs=2) as cp,
            tc.tile_pool(name="pp", bufs=1, space="PSUM") as pp,
            tc.tile_pool(name="dram", bufs=2, space="DRAM") as dp,
        ):
            w0x_s = wp.tile([128, 3 * GL], BF16, tag="w0x", name="w0x")
            nc.sync.dma_start(w0x_s[:], w0x.ap())
            w0h_s = wp.tile([128, KH * GL], BF16, tag="w0h", name="w0h")
            nc.sync.dma_start(w0h_s[:], w0h.ap())
            w1x_s = wp.tile([128, KH * GL], BF16, tag="w1x", name="w1x")
            nc.sync.dma_start(w1x_s[:], w1x.ap())
            w1h_s = wp.tile([128, KH * GL], BF16, tag="w1h", name="w1h")
            nc.sync.dma_start(w1h_s[:], w1h.ap())
            b0_s = wp.tile([HL, 4], F32, tag="b0", name="b0")
            nc.sync.dma_start(b0_s[:], b0d.ap())
            b1_s = wp.tile([HL, 4], F32, tag="b1", name="b1")
            nc.sync.dma_start(b1_s[:], b1d.ap())
            wdec_s = wp.tile([HL, 1], F32R, tag="wdec", name="wdec")
            nc.sync.dma_start(wdec_s[:], wdec.ap())
            z0 = wp.tile([128, Bh], BF16, tag="z0", name="z0")
            nc.vector.memset(z0[:], 0.0)

            def wx(kc, m):
                return w0x_s[0 : DK[kc], kc * GL + m * 128 : kc * GL + (m + 1) * 128]

            def wh(ws, k, m):
                return ws[0:128, k * GL + m * 128 : k * GL + (m + 1) * 128]

            def psap(ps, m):
                return ps[m // 2][:, (m % 2) * Bh : (m % 2 + 1) * Bh]

            # per-half state
            xt = [None] * NH       # x_t tiles [128, 3, Bh]
            h0T = [None] * NH      # gathered h0(t).T [128, KH, Bh]
            h1T = [None] * NH      # gathered h1(t-1).T
            c0p = [None] * NH
            c1p = [None] * NH
            h0b = [None] * NH      # local h0 slice bf16 [128, Bh]
            h1b = [None] * NH
            acc = [None] * NH

            def load_x(h, t):
                xn = xp.tile([128, 3, Bh], BF16, tag=f"xt{h}", name=f"xt{h}")
                nc.sync.dma_start(
                    xn[:, :, :], xTv.ap()[t, :, :, h * Bh : (h + 1) * Bh]
                )
                xt[h] = xn

            def l0_mm(h, t):
                ps0 = [
                    pp.tile([128, 2 * Bh], F32, tag=f"ps0{h}{j}", name=f"ps0{h}{j}")
                    for j in range(2)
                ]
                for m in range(4):
                    out = psap(ps0, m)
                    for kc in range(3):
                        nc.tensor.matmul(
                            out,
                            wx(kc, m),
                            xt[h][0 : DK[kc], kc, :],
                            start=(kc == 0),
                            stop=(t == 0 and kc == 2),
                        )
                    if t > 0:
                        for k in range(KH):
                            nc.tensor.matmul(
                                out,
                                wh(w0h_s, k, m),
                                h0T[h][:, k, :],
                                start=False,
                                stop=(k == KH - 1),
                            )
                return ps0

            def cell0(h, t, ps0):
                zi = zp.tile([128, Bh], F32, tag=f"zi{h}", name=f"zi{h}")
                zf = zp.tile([128, Bh], F32, tag=f"zf{h}", name=f"zf{h}")
                zg = zp.tile([128, Bh], F32, tag=f"zg{h}", name=f"zg{h}")
                zo = zp.tile([128, Bh], F32, tag=f"zo{h}", name=f"zo{h}")
                nc.scalar.activation(zi[:], psap(ps0, 0), AF.Sigmoid, bias=b0_s[:, 0:1])
                if t > 0:
                    nc.scalar.activation(zf[:], psap(ps0, 1), AF.Sigmoid, bias=b0_s[:, 1:2])
                nc.scalar.activation(zg[:], psap(ps0, 2), AF.Tanh, bias=b0_s[:, 2:3])
                nc.scalar.activation(zo[:], psap(ps0, 3), AF.Sigmoid, bias=b0_s[:, 3:4])
                c0 = cp.tile([128, Bh], F32, tag=f"c0{h}", name=f"c0{h}")
                if t == 0:
                    nc.vector.tensor_mul(c0[:], zi[:], zg[:])
                else:
                    ca = zp.tile([128, Bh], F32, tag=f"ca{h}", name=f"ca{h}")
                    cb = zp.tile([128, Bh], F32, tag=f"cb{h}", name=f"cb{h}")
                    nc.vector.tensor_mul(ca[:], zf[:], c0p[h][:])
                    nc.vector.tensor_mul(cb[:], zi[:], zg[:])
                    nc.vector.tensor_add(c0[:], ca[:], cb[:])
                c0p[h] = c0
                tc0 = zp.tile([128, Bh], F32, tag=f"tc0{h}", name=f"tc0{h}")
                nc.scalar.activation(tc0[:], c0[:], AF.Tanh)
                hb = zp.tile([128, Bh], BF16, tag=f"h0b{h}", name=f"h0b{h}")
                nc.vector.tensor_mul(hb[:], zo[:], tc0[:])
                h0b[h] = hb

            def gather(h, t):
                gi = dp.tile([2, 128, Bh], BF16, tag=f"gIn{h}", name=f"gIn{h}")
                nc.sync.dma_start(gi[0], h0b[h][:])
                if t > 0:
                    nc.sync.dma_start(gi[1], h1b[h][:])
                else:
                    nc.sync.dma_start(gi[1], z0[:])
                go = dp.tile(
                    [KH, 2, 128, Bh], BF16, tag=f"gOut{h}", name=f"gOut{h}",
                    addr_space=("Local" if _NO_COLL else "Shared"),
                )
                if not _NO_COLL:
                    nc.gpsimd.collective_compute(
                        "AllGather", ALU.bypass, replica_groups=rg,
                        ins=[gi.opt()], outs=[go.opt()],
                    )
                else:
                    for c in range(NCORES):
                        nc.gpsimd.dma_start(go[c], gi[:])
                return go

            def loads(h, t, go):
                hn = hp.tile([128, KH, Bh], BF16, tag=f"h0T{h}", name=f"h0T{h}")
                nc.sync.dma_start(hn[:, :, :], go[:, 0, :, :].transpose([1, 0, 2]))
                h0T[h] = hn
                if t > 0:
                    hm = hp.tile([128, KH, Bh], BF16, tag=f"h1T{h}", name=f"h1T{h}")
                    nc.sync.dma_start(hm[:, :, :], go[:, 1, :, :].transpose([1, 0, 2]))
                    h1T[h] = hm

            def l1_mm(h, t):
                ps1 = [
                    pp.tile([128, 2 * Bh], F32, tag=f"ps1{h}{j}", name=f"ps1{h}{j}")
                    for j in range(2)
                ]
                for m in range(4):
                    out = psap(ps1, m)
                    if t > 0:
                        for k in range(KH):
                            nc.tensor.matmul(
                                out, wh(w1h_s, k, m), h1T[h][:, k, :],
                                start=(k == 0), stop=False,
                            )
                    for k in range(KH):
                        nc.tensor.matmul(
                            out, wh(w1x_s, k, m), h0T[h][:, k, :],
                            start=(t == 0 and k == 0), stop=(k == KH - 1),
                        )
                return ps1

            def cell1(h, t, ps1):
                yi = zp.tile([128, Bh], F32, tag=f"yi{h}", name=f"yi{h}")
                yf = zp.tile([128, Bh], F32, tag=f"yf{h}", name=f"yf{h}")
                yg = zp.tile([128, Bh], F32, tag=f"yg{h}", name=f"yg{h}")
                yo = zp.tile([128, Bh], F32, tag=f"yo{h}", name=f"yo{h}")
                nc.scalar.activation(yi[:], psap(ps1, 0), AF.Sigmoid, bias=b1_s[:, 0:1])
                if t > 0:
                    nc.scalar.activation(yf[:], psap(ps1, 1), AF.Sigmoid, bias=b1_s[:, 1:2])
                nc.scalar.activation(yg[:], psap(ps1, 2), AF.Tanh, bias=b1_s[:, 2:3])
                nc.scalar.activation(yo[:], psap(ps1, 3), AF.Sigmoid, bias=b1_s[:, 3:4])
                c1 = cp.tile([128, Bh], F32, tag=f"c1{h}", name=f"c1{h}")
                if t == 0:
                    nc.vector.tensor_mul(c1[:], yi[:], yg[:])
                else:
                    da = zp.tile([128, Bh], F32, tag=f"da{h}", name=f"da{h}")
                    db = zp.tile([128, Bh], F32, tag=f"db{h}", name=f"db{h}")
                    nc.vector.tensor_mul(da[:], yf[:], c1p[h][:])
                    nc.vector.tensor_mul(db[:], yi[:], yg[:])
                    nc.vector.tensor_add(c1[:], da[:], db[:])
                c1p[h] = c1
                tc1 = zp.tile([128, Bh], F32, tag=f"tc1{h}", name=f"tc1{h}")
                nc.scalar.activation(tc1[:], c1[:], AF.Tanh)
                hf = zp.tile([128, Bh], F32R, tag=f"h1f{h}", name=f"h1f{h}")
                nc.vector.tensor_mul(hf[:], yo[:], tc1[:])
                if t < t_steps - 1:
                    hb = zp.tile([128, Bh], BF16, tag=f"h1b{h}", name=f"h1b{h}")
                    nc.scalar.copy(hb[:], hf[:])
                    h1b[h] = hb
                if t == 0:
                    a = cp.tile([128, Bh], F32R, tag=f"acc{h}", name=f"acc{h}")
                    nc.vector.tensor_copy(a[:], hf[:])
                else:
                    a = cp.tile([128, Bh], F32R, tag=f"acc{h}", name=f"acc{h}")
                    nc.vector.tensor_add(a[:], acc[h][:], hf[:])
                acc[h] = a

            # ---- prologue: step 0 layer0 + first gathers ----
            for h in range(NH):
                load_x(h, 0)
            gouts = [None] * NH
            for h in range(NH):
                ps0 = l0_mm(h, 0)
                cell0(h, 0, ps0)
                gouts[h] = gather(h, 0)

            # ---- main loop ----
            for i in range(t_steps):
                for h in range(NH):
                    if i + 1 < t_steps:
                        load_x(h, i + 1)
                for h in range(NH):
                    loads(h, i, gouts[h])
                for h in range(NH):
                    ps1 = l1_mm(h, i)
                    cell1(h, i, ps1)
                    if i + 1 < t_steps:
                        ps0 = l0_mm(h, i + 1)
                        cell0(h, i + 1, ps0)
                        gouts[h] = gather(h, i + 1)

            # ---- decoder: out_p = (acc/T) . wdec per half (host sums cores) ----
            psd = pp.tile([128, 2 * Bh], F32, tag="ps000", name="psd")
            for h in range(NH):
                nc.tensor.matmul(
                    psd[0:1, h * Bh : (h + 1) * Bh], wdec_s[:, 0:1], acc[h][:],
                    start=True, stop=True,
                )
            outt = zp.tile([1, B], F32, tag="outt", name="outt")
            nc.scalar.copy(outt[:], psd[0:1, :])
            nc.sync.dma_start(out_p.ap(), outt[:])

    nc.compile()
    return nc


def _prep_inputs(x, W_ih0, W_hh0, b_ih0, b_hh0, W_ih1, W_hh1, b_ih1, b_hh1, W_dec, t_steps, t_total=None):
    import ml_dtypes

    bf16 = ml_dtypes.bfloat16
    t_total = t_total or t_steps
    xT = np.transpose(x[:, :t_total, :], (1, 2, 0)).astype(bf16)  # [T, D, B]
    xTv = np.zeros((t_total, 128, 3, B), bf16)
    for kc in range(3):
        xTv[:, 0 : DK[kc], kc, :] = xT[:, kc * 128 : kc * 128 + DK[kc], :]
    b0 = (b_ih0 + b_hh0).astype(np.float32)
    b1 = (b_ih1 + b_hh1).astype(np.float32)
    in_maps = []
    for c in range(NCORES):
        rows = np.concatenate([g * H + np.arange(c * HL, (c + 1) * HL) for g in range(4)])

        def pack(W, nk):
            Wt = np.ascontiguousarray(W[rows, :].T.astype(np.float32))  # [K_total, GL]
            arr = np.zeros((128, nk * GL), np.float32)
            for k in range(nk):
                kp = min(128, Wt.shape[0] - k * 128)
                arr[0:kp, k * GL : k * GL + GL] = Wt[k * 128 : k * 128 + kp, :]
            return arr.astype(bf16)

        in_maps.append({
            "xTv": xTv,
            "w0x": pack(W_ih0, 3),
            "w0h": pack(W_hh0, KH),
            "w1x": pack(W_ih1, KH),
            "w1h": pack(W_hh1, KH),
            "b0d": np.ascontiguousarray(b0[rows].reshape(4, HL).T),
            "b1d": np.ascontiguousarray(b1[rows].reshape(4, HL).T),
            "wdec": np.ascontiguousarray(
                (W_dec[0, c * HL : (c + 1) * HL] / np.float32(t_steps)).reshape(HL, 1)
            ).astype(np.float32),
        })
    return in_maps


def _run(inputs, t_steps, **spmd_kwargs):
    nc = _build(t_steps)
    in_maps = _prep_inputs(
        inputs["x"], inputs["W_ih0"], inputs["W_hh0"], inputs["b_ih0"], inputs["b_hh0"],
        inputs["W_ih1"], inputs["W_hh1"], inputs["b_ih1"], inputs["b_hh1"], inputs["W_dec"],
        t_steps,
    )
    res = run_bass_kernel_spmd(nc, in_maps, core_ids=list(range(NCORES)), **spmd_kwargs)
    part = sum(res.results[c]["out_p"][0] for c in range(NCORES))
    out = (part + inputs["b_dec"][0]).astype(np.float32).reshape(B, 1)
    return out, res


def kernel(**inputs):
    out, _ = _run(inputs, T)
    return out


# revision 6
# speedup vs baseline: 1.6571x; 1.0691x over previous
"""Trainium2 Bass kernel for a 2-layer FC-LSTM (B=512, T=128, D=300, H=1024).

Strategy: model-parallel over the hidden dim (each of 8 cores owns 128
hidden units per layer = 512 gate rows), with the batch processed as two
independent 256-sample halves that are software-pipelined against each
other. Weights and activations are bf16 (fp32 PSUM accumulation and fp32
cell state), so matmuls run at the PE's full 1 cycle/row rate.

Per half and step a single fused AllGather moves [h0(t); h1(t-1)] through
a Shared-output HBM buffer; it is issued ~one full iteration before its
consumers, so collective latency hides under the other half's matmuls.
All data DMAs ride the sync-engine hardware DGE queue (fused 3D access
patterns: one DMA per x-step prefetch, one per gathered h tensor).
"""
import sys

sys.path.insert(0, "/opt/trn_rl_repo")

import os
import numpy as np

import concourse.bass as bass
import concourse.bacc as bacc
import concourse.mybir as mybir
from concourse import tile
from concourse.bass_utils import run_bass_kernel_spmd

B, T, D, H = 512, 128, 300, 1024
NCORES = 8
NH = 2                    # batch halves (independent recurrences)
Bh = B // NH              # 256 batch per half
HL = H // NCORES          # 128 hidden units owned per core (per layer)
GL = 4 * HL               # 512 gate rows owned per core
DK = [128, 128, 44]       # D=300 split into K-chunks
KH = H // 128             # 8 K-chunks over the hidden dim

F32 = mybir.dt.float32
F32R = mybir.dt.float32r
BF16 = mybir.dt.bfloat16
AF = mybir.ActivationFunctionType
ALU = mybir.AluOpType
_NO_COLL = bool(os.environ.get("KERNEL_NO_COLL"))


def _build(t_steps, t_total=None):
    t_total = t_total or t_steps
    nc = bacc.Bacc("TRN2", target_bir_lowering=False, debug=False, num_devices=NCORES)

    # x pre-transposed+padded on host: xTv[t, p, kc, b] = x[b, t, kc*128+p]
    xTv = nc.dram_tensor("xTv", [t_total, 128, 3, B], BF16, kind="ExternalInput")
    w0x = nc.dram_tensor("w0x", [128, 3 * GL], BF16, kind="ExternalInput")
    w0h = nc.dram_tensor("w0h", [128, KH * GL], BF16, kind="ExternalInput")
    w1x = nc.dram_tensor("w1x", [128, KH * GL], BF16, kind="ExternalInput")
    w1h = nc.dram_tensor("w1h", [128, KH * GL], BF16, kind="ExternalInput")
    b0d = nc.dram_tensor("b0d", [HL, 4], F32, kind="ExternalInput")
    b1d = nc.dram_tensor("b1d", [HL, 4], F32, kind="ExternalInput")
    wdec = nc.dram_tensor("wdec", [HL, 1], F32R, kind="ExternalInput")
    out_p = nc.dram_tensor("out_p", [1, B], F32, kind="ExternalOutput")

    rg = [list(range(NCORES))]

    with tile.TileContext(nc) as tc:
        with (
            tc.tile_pool(name="wpool", bufs=1) as wp,
            tc.tile_pool(name="xpool", bufs=2) as xp,
            tc.tile_pool(name="hpool", bufs=2) as hp,
            tc.tile_pool(name="zpool", bufs=2) as zp,
            tc.tile_pool(name="cpool", bufs=2) as cp,
            tc.tile_pool(name="pp", bufs=1, space="PSUM") as pp,
            tc.tile_pool(name="dram", bufs=2, space="DRAM") as dp,
        ):
            w0x_s = wp.tile([128, 3 * GL], BF16, tag="w0x", name="w0x")
            nc.sync.dma_start(w0x_s[:], w0x.ap())
            w0h_s = wp.tile([128, KH * GL], BF16, tag="w0h", name="w0h")
            nc.sync.dma_start(w0h_s[:], w0h.ap())
            w1x_s = wp.tile([128, KH * GL], BF16, tag="w1x", name="w1x")
            nc.sync.dma_start(w1x_s[:], w1x.ap())
            w1h_s = wp.tile([128, KH * GL], BF16, tag="w1h", name="w1h")
            nc.sync.dma_start(w1h_s[:], w1h.ap())
            b0_s = wp.tile([HL, 4], F32, tag="b0", name="b0")
            nc.sync.dma_start(b0_s[:], b0d.ap())
            b1_s = wp.tile([HL, 4], F32, tag="b1", name="b1")
            nc.sync.dma_start(b1_s[:], b1d.ap())
            wdec_s = wp.tile([HL, 1], F32R, tag="wdec", name="wdec")
            nc.sync.dma_start(wdec_s[:], wdec.ap())
            z0 = wp.tile([128, Bh], BF16, tag="z0", name="z0")
            nc.vector.memset(z0[:], 0.0)

            def wx(kc, m):
                return w0x_s[0 : DK[kc], kc * GL + m * 128 : kc * GL + (m + 1) * 128]

            def wh(ws, k, m):
                return ws[0:128, k * GL + m * 128 : k * GL + (m + 1) * 128]

            def psap(ps, m):
                return ps[m // 2][:, (m % 2) * Bh : (m % 2 + 1) * Bh]

            # per-half state
            xt = [None] * NH       # x_t tiles [128, 3, Bh]
            h0T = [None] * NH      # gathered h0(t).T [128, KH, Bh]
            h1T = [None] * NH      # gathered h1(t-1).T
            c0p = [None] * NH
            c1p = [None] * NH
            h0b = [None] * NH      # local h0 slice bf16 [128, Bh]
            h1b = [None] * NH
            acc = [None] * NH

            def load_x(h, t):
                xn = xp.tile([128, 3, Bh], BF16, tag=f"xt{h}", name=f"xt{h}")
                nc.sync.dma_start(
                    xn[:, :, :], xTv.ap()[t, :, :, h * Bh : (h + 1) * Bh]
                )
                xt[h] = xn

            def l0_mm(h, t):
                ps0 = [
                    pp.tile([128, 2 * Bh], F32, tag=f"ps0{h}{j}", name=f"ps0{h}{j}")
                    for j in range(2)
                ]
                for m in range(4):
                    out = psap(ps0, m)
                    for kc in range(3):
                        nc.tensor.matmul(
                            out,
                            wx(kc, m),
                            xt[h][0 : DK[kc], kc, :],
                            start=(kc == 0),
                            stop=(t == 0 and kc == 2),
                        )
                    if t > 0:
                        for k in range(KH):
                            nc.tensor.matmul(
                                out,
                                wh(w0h_s, k, m),
                                h0T[h][:, k, :],
                                start=False,
                                stop=(k == KH - 1),
                            )
                return ps0

            def cell0(h, t, ps0):
                zi = zp.tile([128, Bh], F32, tag=f"zi{h}", name=f"zi{h}")
                zf = zp.tile([128, Bh], F32, tag=f"zf{h}", name=f"zf{h}")
                zg = zp.tile([128, Bh], F32, tag=f"zg{h}", name=f"zg{h}")
                zo = zp.tile([128, Bh], F32, tag=f"zo{h}", name=f"zo{h}")
                nc.scalar.activation(zi[:], psap(ps0, 0), AF.Sigmoid, bias=b0_s[:, 0:1])
                if t > 0:
                    nc.scalar.activation(zf[:], psap(ps0, 1), AF.Sigmoid, bias=b0_s[:, 1:2])
                nc.scalar.activation(zg[:], psap(ps0, 2), AF.Tanh, bias=b0_s[:, 2:3])
                nc.scalar.activation(zo[:], psap(ps0, 3), AF.Sigmoid, bias=b0_s[:, 3:4])
                c0 = cp.tile([128, Bh], F32, tag=f"c0{h}", name=f"c0{h}")
                if t == 0:
                    nc.vector.tensor_mul(c0[:], zi[:], zg[:])
                else:
                    ca = zp.tile([128, Bh], F32, tag=f"ca{h}", name=f"ca{h}")
                    cb = zp.tile([128, Bh], F32, tag=f"cb{h}", name=f"cb{h}")
                    nc.vector.tensor_mul(ca[:], zf[:], c0p[h][:])
                    nc.vector.tensor_mul(cb[:], zi[:], zg[:])
                    nc.vector.tensor_add(c0[:], ca[:], cb[:])
                c0p[h] = c0
                tc0 = zp.tile([128, Bh], F32, tag=f"tc0{h}", name=f"tc0{h}")
                nc.scalar.activation(tc0[:], c0[:], AF.Tanh)
                hb = zp.tile([128, Bh], BF16, tag=f"h0b{h}", name=f"h0b{h}")
                nc.vector.tensor_mul(hb[:], zo[:], tc0[:])
                h0b[h] = hb

            def gather(h, t):
                gi = dp.tile([2, 128, Bh], BF16, tag=f"gIn{h}", name=f"gIn{h}")
                nc.scalar.dma_start(gi[0], h0b[h][:])
                if t > 0:
                    nc.scalar.dma_start(gi[1], h1b[h][:])
                else:
                    nc.scalar.dma_start(gi[1], z0[:])
                go = dp.tile(
                    [KH, 2, 128, Bh], BF16, tag=f"gOut{h}", name=f"gOut{h}",
                    addr_space=("Local" if _NO_COLL else "Shared"),
                )
                if not _NO_COLL:
                    nc.gpsimd.collective_compute(
                        "AllGather", ALU.bypass, replica_groups=rg,
                        ins=[gi.opt()], outs=[go.opt()],
                    )
                else:
                    for c in range(NCORES):
                        nc.gpsimd.dma_start(go[c], gi[:])
                return go

            def loads(h, t, go):
                KS = KH // 2
                hn = hp.tile([128, KH, Bh], BF16, tag=f"h0T{h}", name=f"h0T{h}")
                hm = None
                if t > 0:
                    hm = hp.tile([128, KH, Bh], BF16, tag=f"h1T{h}", name=f"h1T{h}")
                    for s in range(2):
                        nc.sync.dma_start(
                            hm[:, s * KS : (s + 1) * KS, :],
                            go[s * KS : (s + 1) * KS, 1, :, :].transpose([1, 0, 2]),
                        )
                    h1T[h] = hm
                for s in range(2):
                    nc.sync.dma_start(
                        hn[:, s * KS : (s + 1) * KS, :],
                        go[s * KS : (s + 1) * KS, 0, :, :].transpose([1, 0, 2]),
                    )
                h0T[h] = hn

            def l1_mm(h, t):
                ps1 = [
                    pp.tile([128, 2 * Bh], F32, tag=f"ps1{h}{j}", name=f"ps1{h}{j}")
                    for j in range(2)
                ]
                for m in range(4):
                    out = psap(ps1, m)
                    if t > 0:
                        for k in range(KH):
                            nc.tensor.matmul(
                                out, wh(w1h_s, k, m), h1T[h][:, k, :],
                                start=(k == 0), stop=False,
                            )
                    for k in range(KH):
                        nc.tensor.matmul(
                            out, wh(w1x_s, k, m), h0T[h][:, k, :],
                            start=(t == 0 and k == 0), stop=(k == KH - 1),
                        )
                return ps1

            def cell1(h, t, ps1):
                yi = zp.tile([128, Bh], F32, tag=f"yi{h}", name=f"yi{h}")
                yf = zp.tile([128, Bh], F32, tag=f"yf{h}", name=f"yf{h}")
                yg = zp.tile([128, Bh], F32, tag=f"yg{h}", name=f"yg{h}")
                yo = zp.tile([128, Bh], F32, tag=f"yo{h}", name=f"yo{h}")
                nc.scalar.activation(yi[:], psap(ps1, 0), AF.Sigmoid, bias=b1_s[:, 0:1])
                if t > 0:
                    nc.scalar.activation(yf[:], psap(ps1, 1), AF.Sigmoid, bias=b1_s[:, 1:2])
                nc.scalar.activation(yg[:], psap(ps1, 2), AF.Tanh, bias=b1_s[:, 2:3])
                nc.scalar.activation(yo[:], psap(ps1, 3), AF.Sigmoid, bias=b1_s[:, 3:4])
                c1 = cp.tile([128, Bh], F32, tag=f"c1{h}", name=f"c1{h}")
                if t == 0:
                    nc.vector.tensor_mul(c1[:], yi[:], yg[:])
                else:
                    da = zp.tile([128, Bh], F32, tag=f"da{h}", name=f"da{h}")
                    db = zp.tile([128, Bh], F32, tag=f"db{h}", name=f"db{h}")
                    nc.vector.tensor_mul(da[:], yf[:], c1p[h][:])
                    nc.vector.tensor_mul(db[:], yi[:], yg[:])
                    nc.vector.tensor_add(c1[:], da[:], db[:])
                c1p[h] = c1
                tc1 = zp.tile([128, Bh], F32, tag=f"tc1{h}", name=f"tc1{h}")
                nc.scalar.activation(tc1[:], c1[:], AF.Tanh)
                hf = zp.tile([128, Bh], F32R, tag=f"h1f{h}", name=f"h1f{h}")
                nc.vector.tensor_mul(hf[:], yo[:], tc1[:])
                if t < t_steps - 1:
                    hb = zp.tile([128, Bh], BF16, tag=f"h1b{h}", name=f"h1b{h}")
                    nc.scalar.copy(hb[:], hf[:])
                    h1b[h] = hb
                if t == 0:
                    a = cp.tile([128, Bh], F32R, tag=f"acc{h}", name=f"acc{h}")
                    nc.vector.tensor_copy(a[:], hf[:])
                else:
                    a = cp.tile([128, Bh], F32R, tag=f"acc{h}", name=f"acc{h}")
                    nc.vector.tensor_add(a[:], acc[h][:], hf[:])
                acc[h] = a

            # ---- prologue: step 0 layer0 + first gathers + first loads ----
            for h in range(NH):
                load_x(h, 0)
            gouts = [None] * NH
            for h in range(NH):
                ps0 = l0_mm(h, 0)
                cell0(h, 0, ps0)
                gouts[h] = gather(h, 0)
            for h in range(NH):
                if t_steps > 1:
                    load_x(h, 1)
            for h in range(NH):
                loads(h, 0, gouts[h])

            # ---- main loop (loads/x for step i+1 issued at end of iter i) ----
            for i in range(t_steps):
                for h in range(NH):
                    ps1 = l1_mm(h, i)
                    cell1(h, i, ps1)
                    if i + 1 < t_steps:
                        ps0 = l0_mm(h, i + 1)
                        cell0(h, i + 1, ps0)
                        gouts[h] = gather(h, i + 1)
                for h in range(NH):
                    if i + 2 < t_steps:
                        load_x(h, i + 2)
                for h in range(NH):
                    if i + 1 < t_steps:
                        loads(h, i + 1, gouts[h])

            # ---- decoder: out_p = (acc/T) . wdec per half (host sums cores) ----
            psd = pp.tile([128, 2 * Bh], F32, tag="ps000", name="psd")
            for h in range(NH):
                nc.tensor.matmul(
                    psd[0:1, h * Bh : (h + 1) * Bh], wdec_s[:, 0:1], acc[h][:],
                    start=True, stop=True,
                )
            outt = zp.tile([1, B], F32, tag="outt", name="outt")
            nc.scalar.copy(outt[:], psd[0:1, :])
            nc.sync.dma_start(out_p.ap(), outt[:])

    nc.compile()
    return nc


def _prep_inputs(x, W_ih0, W_hh0, b_ih0, b_hh0, W_ih1, W_hh1, b_ih1, b_hh1, W_dec, t_steps, t_total=None):
    import ml_dtypes

    bf16 = ml_dtypes.bfloat16
    t_total = t_total or t_steps
    xT = np.transpose(x[:, :t_total, :], (1, 2, 0)).astype(bf16)  # [T, D, B]
    xTv = np.zeros((t_total, 128, 3, B), bf16)
    for kc in range(3):
        xTv[:, 0 : DK[kc], kc, :] = xT[:, kc * 128 : kc * 128 + DK[kc], :]
    b0 = (b_ih0 + b_hh0).astype(np.float32)
    b1 = (b_ih1 + b_hh1).astype(np.float32)
    in_maps = []
    for c in range(NCORES):
        rows = np.concatenate([g * H + np.arange(c * HL, (c + 1) * HL) for g in range(4)])

        def pack(W, nk):
            Wt = np.ascontiguousarray(W[rows, :].T.astype(np.float32))  # [K_total, GL]
            arr = np.zeros((128, nk * GL), np.float32)
            for k in range(nk):
                kp = min(128, Wt.shape[0] - k * 128)
                arr[0:kp, k * GL : k * GL + GL] = Wt[k * 128 : k * 128 + kp, :]
            return arr.astype(bf16)

        in_maps.append({
            "xTv": xTv,
            "w0x": pack(W_ih0, 3),
            "w0h": pack(W_hh0, KH),
            "w1x": pack(W_ih1, KH),
            "w1h": pack(W_hh1, KH),
            "b0d": np.ascontiguousarray(b0[rows].reshape(4, HL).T),
            "b1d": np.ascontiguousarray(b1[rows].reshape(4, HL).T),
            "wdec": np.ascontiguousarray(
                (W_dec[0, c * HL : (c + 1) * HL] / np.float32(t_steps)).reshape(HL, 1)
            ).astype(np.float32),
        })
    return in_maps


def _run(inputs, t_steps, **spmd_kwargs):
    nc = _build(t_steps)
    in_maps = _prep_inputs(
        inputs["x"], inputs["W_ih0"], inputs["W_hh0"], inputs["b_ih0"], inputs["b_hh0"],
        inputs["W_ih1"], inputs["W_hh1"], inputs["b_ih1"], inputs["b_hh1"], inputs["W_dec"],
        t_steps,
    )
    res = run_bass_kernel_spmd(nc, in_maps, core_ids=list(range(NCORES)), **spmd_kwargs)
    part = sum(res.results[c]["out_p"][0] for c in range(NCORES))
    out = (part + inputs["b_dec"][0]).astype(np.float32).reshape(B, 1)
    return out, res


def kernel(**inputs):
    out, _ = _run(inputs, T)
    return out
